# revision 1
# baseline (speedup 1.0000x reference)
"""Causal attention kernel for Trainium2, 8-core SPMD (v9: DR scores, pair-shared stationaries, causal AV skip).

Interleaved-key split: 8 cores = 4 batches x 2 key-shards.  Core (b, h)
handles key tiles kt === h (mod 2) of batch b (2048 keys) but ALL 4096
queries, producing unnormalized partial attention sums + partial softmax
denominators; the host merges: out = (avA + avB) / (lA + lB).

v2 changes vs v1:
- All matmul operands bf16 (inputs pre-cast on host).  PSUM stays f32.
- K^T kept resident in SBUF (no DRAM round-trip; v1 re-read ~37 MB).
- No max-subtraction in softmax (|scores*scale| <~ 3), partials merge
  exactly on the host.

Per-core causal structure: for query chunk c (512 q), local key tiles
kt' = 0..2c+1 are active; the last two are diagonal and get additive
masks passed as per-core input data (mask_in[2,128,512]).
"""
import numpy as np
import ml_dtypes

import concourse.bacc as bacc
import concourse.mybir as mybir
import concourse.tile as tile
from concourse.bass_utils import run_bass_kernel_spmd

F32 = mybir.dt.float32
BF16 = mybir.dt.bfloat16
FP8 = mybir.dt.float8e4
DR = mybir.MatmulPerfMode.DoubleRow
EXP = mybir.ActivationFunctionType.Exp

B, N, E, O = 4, 4096, 1024, 1024
NE, NO = E // 128, O // 128
NK = N // 2                          # local keys per core (2048)
NKT_L = NK // 128                    # 16 local k-tiles
QC = 512
NCHUNK = N // QC                     # 8
SCALE = 1.0 / 32.0
MASKVAL = -1e9


def _emit(nc, tc, xT, xkT, WqT, WkT, WvT, mask_in, out_u, l_out):
    xT3 = xT.rearrange("(a p) n -> a p n", p=128)
    xk3 = xkT.rearrange("(a p) n -> a p n", p=128)
    WqT3 = WqT.rearrange("(a p) n -> a p n", p=128)
    WkT3 = WkT.rearrange("(a p) n -> a p n", p=128)
    WvT3 = WvT.rearrange("(a p) n -> a p n", p=128)
    out3 = out_u.rearrange("(a p) n -> a p n", p=128)

    # outer pool: resident K^T, V, Wq, masks (live across both phases)
    from contextlib import ExitStack
    _stk = ExitStack()
    outer = _stk.enter_context(tc.tile_pool(name="outer", bufs=1))
    vres = [outer.tile([128, O], BF16, tag=f"vres{k}", name=f"vres_{k}")
            for k in range(NKT_L)]
    # K^T resident as fp8 DoubleRow pairs: per (o-pair i, k-tile kt) one
    # CONTIGUOUS [128, 2, 128] stationary tile (j = o-tile 2i+j)
    kres = [[outer.tile([128, 2, 128], FP8, tag=f"kres{i}_{kt}",
                        name=f"kres_{i}_{kt}") for kt in range(NKT_L)]
            for i in range(NO // 2)]
    wq, masks = [], []
    # ---------------- phase 1: K^T (local keys) and V projections ------------
    with tc.tile_pool(name="p1", bufs=1) as sb, \
         tc.tile_pool(name="p1p", bufs=1, space="PSUM") as pp:
        # critical-path loads first: wk + x feed the first K-proj matmuls;
        # wv/wq/masks are needed later.  All 2048 local keys resident at
        # once so each wk stationary serves 4 moving blocks.
        wk, wv, xe = [], [], []
        for e in range(NE):
            wkt = sb.tile([128, O], BF16, tag=f"wk{e}", name=f"wk_{e}")
            nc.sync.dma_start(wkt, WkT3[e])
            wk.append(wkt)
            xet = sb.tile([128, NK], BF16, tag=f"xe{e}", name=f"xe_{e}")
            nc.sync.dma_start(xet, xk3[e])
            xe.append(xet)
        for e in range(NE):
            wvt = sb.tile([128, O], BF16, tag=f"wv{e}", name=f"wv_{e}")
            nc.sync.dma_start(wvt, WvT3[e])
            wv.append(wvt)
        for e in range(NE):
            wqt = outer.tile([128, O], BF16, tag=f"wq{e}", name=f"wq_{e}")
            nc.sync.dma_start(wqt, WqT3[e])
            wq.append(wqt)
        for i in range(2):
            m = outer.tile([128, QC], F32, tag=f"mask{i}", name=f"mask_{i}")
            nc.sync.dma_start(m, mask_in[i])
            masks.append(m)
        for o in range(NO):
            pk = [pp.tile([128, 512], F32, tag="pp", bufs=8,
                          name=f"pk_{o}_{kc}") for kc in range(NK // 512)]
            for e in range(NE):
                for kc in range(NK // 512):
                    nc.tensor.matmul(
                        pk[kc], wk[e][:, o * 128:(o + 1) * 128],
                        xe[e][:, kc * 512:(kc + 1) * 512],
                        start=(e == 0), stop=(e == NE - 1))
            for kc in range(NK // 512):
                for t in range(4):
                    nc.scalar.copy(
                        kres[o // 2][kc * 4 + t][:, o % 2, :],
                        pk[kc][:, t * 128:(t + 1) * 128])
        for ns in range(NKT_L):
            pvs = [pp.tile([128, 512], F32, tag="pp", bufs=8,
                           name=f"pv_{ns}_{ovc}") for ovc in range(2)]
            for e in range(NE):
                for ovc in range(2):
                    nc.tensor.matmul(
                        pvs[ovc], xe[e][:, ns * 128:(ns + 1) * 128],
                        wv[e][:, ovc * 512:(ovc + 1) * 512],
                        start=(e == 0), stop=(e == NE - 1))
            for ovc in range(2):
                nc.scalar.copy(
                    vres[ns][:, ovc * 512:(ovc + 1) * 512], pvs[ovc])

    # ---------------- phase 2: attention ----------------
    with tc.tile_pool(name="p2", bufs=1) as sb, \
         tc.tile_pool(name="p2p", bufs=1, space="PSUM") as pp:
        ones = sb.tile([128, 1], F32, tag="ones", name="ones")
        nc.gpsimd.memset(ones, 1.0)

        # chunk pairs, biggest first: shared Q-proj stationaries serve
        # both chunks; big chunks' output DMA overlaps later compute
        for cp in range(NCHUNK // 2):
            cA = NCHUNK - 1 - 2 * cp
            pair = ((0, cA), (1, cA - 1))
            xc, qt = {}, {}
            for par, c in pair:
                xc[par] = []
                for e in range(NE):
                    xct = sb.tile([128, QC], BF16, tag=f"xc{e}_{par}",
                                  bufs=1, name=f"xc_{c}_{e}")
                    nc.sync.dma_start(
                        xct, xT3[e][:, c * QC:(c + 1) * QC])
                    xc[par].append(xct)
                qt[par] = [sb.tile([128, 2, QC], FP8, tag=f"qt{i}_{par}",
                                   bufs=1, name=f"qt_{c}_{i}")
                           for i in range(NO // 2)]
            for oi in range(NO):
                qps = {par: pp.tile([128, QC], F32, tag="avqp", bufs=3,
                                    name=f"qps_{c}_{oi}")
                       for par, c in pair}
                for e in range(NE):
                    for par, c in pair:
                        nc.tensor.matmul(
                            qps[par], wq[e][:, oi * 128:(oi + 1) * 128],
                            xc[par][e],
                            start=(e == 0), stop=(e == NE - 1))
                for par, c in pair:
                    nc.scalar.copy(qt[par][oi // 2][:, oi % 2, :], qps[par])

            # scores for BOTH chunks interleaved so each kres stationary
            # serves the pair; pts kept per-parity
            pts = {}
            lacc = {}
            for par, c in pair:
                pts[par] = []
                lacc[par] = sb.tile([128, QC], F32, tag=f"lacc{par}",
                                    bufs=1, name=f"lacc_{c}")
            nktA = 2 * cA + 2
            for kt in range(nktA):
                sps = {}
                for par, c in pair:
                    if kt >= 2 * c + 2:
                        continue
                    sps[par] = pp.tile([128, QC], F32, tag="s", bufs=2,
                                       name=f"sps_{c}_{kt}")
                for i in range(NO // 2):
                    for par in sps:
                        nc.tensor.matmul(
                            sps[par], kres[i][kt], qt[par][i],
                            start=(i == 0), stop=(i == NO // 2 - 1),
                            perf_mode=DR)
                for par, c in pair:
                    if par not in sps:
                        continue
                    nkt = 2 * c + 2
                    di = kt - (nkt - 2)
                    if di >= 0:
                        nc.vector.tensor_add(sps[par], sps[par], masks[di])
                    pt = sb.tile([128, QC], BF16, tag=f"pt{kt}_{par}",
                                 bufs=1, name=f"pt_{c}_{kt}")
                    nc.scalar.activation(pt, sps[par], EXP, scale=SCALE)
                    pts[par].append(pt)
                    if kt == 0:
                        nc.vector.tensor_copy(lacc[par], pt)
                    else:
                        nc.vector.tensor_add(lacc[par], lacc[par], pt)

            for par, c in pair:
                _attend(nc, sb, pp, vres, pts[par], lacc[par], ones,
                        out3, l_out, c)
    _stk.close()


def _attend(nc, sb, pp, vres, pts, lacc, ones, out3, l_out, c):
    nkt = 2 * c + 2
    if True:
        if True:
            lps = pp.tile([1, QC], F32, tag="l", bufs=1, name=f"lps_{c}")

            # AV: s-major with ovc inner so each pt stationary serves both
            # o-halves; q-subtiles s<2 of the last (diagonal) k-tile are
            # fully causal-masked for either key parity — skip exactly.
            for s in range(4):
                last = nkt - 2 if s < 2 else nkt - 1
                av = [pp.tile([128, 512], F32, tag="av2", bufs=2,
                              name=f"av_{c}_{s}_{ovc}") for ovc in range(2)]
                for kt in range(last + 1):
                    for ovc in range(2):
                        nc.tensor.matmul(
                            av[ovc], pts[kt][:, s * 128:(s + 1) * 128],
                            vres[kt][:, ovc * 512:(ovc + 1) * 512],
                            start=(kt == 0), stop=(kt == last))
                for ovc in range(2):
                    ot = sb.tile([128, 512], F32, tag="ot", bufs=4,
                                 name=f"ot_{c}_{s}_{ovc}")
                    if ovc == 0:
                        nc.scalar.copy(ot, av[ovc])
                    else:
                        nc.vector.tensor_copy(ot, av[ovc])
                    nc.sync.dma_start(
                        out3[c * 4 + s][:, ovc * 512:(ovc + 1) * 512], ot)

            nc.tensor.matmul(lps, ones, lacc, start=True, stop=True)
            lt = sb.tile([1, QC], F32, tag="lt", bufs=2, name=f"lt_{c}")
            nc.scalar.copy(lt, lps)
            nc.sync.dma_start(l_out[c:c + 1, :], lt)


_NC_CACHE = None


def build_program():
    global _NC_CACHE
    if _NC_CACHE is not None:
        return _NC_CACHE
    nc = bacc.Bacc("TRN2", target_bir_lowering=False, debug=False)
    xT = nc.dram_tensor("xT", [E, N], BF16, kind="ExternalInput").ap()
    xkT = nc.dram_tensor("xkT", [E, NK], BF16, kind="ExternalInput").ap()
    WqT = nc.dram_tensor("WqT", [E, O], BF16, kind="ExternalInput").ap()
    WkT = nc.dram_tensor("WkT", [E, O], BF16, kind="ExternalInput").ap()
    WvT = nc.dram_tensor("WvT", [E, O], BF16, kind="ExternalInput").ap()
    mask_in = nc.dram_tensor("mask_in", [2, 128, QC], F32,
                             kind="ExternalInput").ap()
    out_u = nc.dram_tensor("out_u", [N, O], F32, kind="ExternalOutput").ap()
    l_out = nc.dram_tensor("l_out", [NCHUNK, QC], F32,
                           kind="ExternalOutput").ap()
    with tile.TileContext(nc) as tc:
        _emit(nc, tc, xT, xkT, WqT, WkT, WvT, mask_in, out_u, l_out)
    nc.compile()
    _NC_CACHE = nc
    return nc


def make_in_maps(x, Wq, Wk, Wv):
    bf = ml_dtypes.bfloat16
    x = np.asarray(x, np.float32)
    WqT = np.ascontiguousarray(np.asarray(Wq, np.float32).T.astype(bf))
    WkT = np.ascontiguousarray(np.asarray(Wk, np.float32).T.astype(bf))
    WvT = np.ascontiguousarray(np.asarray(Wv, np.float32).T.astype(bf))
    kk = np.arange(128)[:, None]
    qq = np.arange(QC)[None, :]
    in_maps = []
    for c in range(8):
        b, h = divmod(c, 2)
        xb = x[b]
        xk = xb.reshape(N // 128, 128, E)[h::2].reshape(NK, E)
        masks = np.stack([
            np.where(qq >= (2 * i + h) * 128 + kk, 0.0, MASKVAL)
            for i in range(2)
        ]).astype(np.float32)
        in_maps.append({
            "xT": np.ascontiguousarray(xb.T.astype(bf)),
            "xkT": np.ascontiguousarray(xk.T.astype(bf)),
            "WqT": WqT,
            "WkT": WkT,
            "WvT": WvT,
            "mask_in": masks,
        })
    return in_maps


def gather_out(results):
    out = np.empty((B, N, O), np.float32)
    for b in range(B):
        a0 = results[2 * b]["out_u"].astype(np.float64)
        a1 = results[2 * b + 1]["out_u"].astype(np.float64)
        l0 = results[2 * b]["l_out"].astype(np.float64).reshape(N, 1)
        l1 = results[2 * b + 1]["l_out"].astype(np.float64).reshape(N, 1)
        out[b] = ((a0 + a1) / (l0 + l1)).astype(np.float32)
    return out


def kernel(x, Wq, Wk, Wv, **run_kwargs):
    nc = build_program()
    in_maps = make_in_maps(x, Wq, Wk, Wv)
    res = run_bass_kernel_spmd(nc, in_maps, core_ids=list(range(8)),
                               **run_kwargs)
    out = gather_out(res.results)
    if run_kwargs:
        return out, res
    return out



# revision 4
# speedup vs baseline: 1.4063x; 1.4063x over previous
"""Causal attention kernel for Trainium2, 8-core SPMD (v11: v9 + half-width
diagonal score tiles + double-buffered pts for cross-pair overlap).

Interleaved-key split: 8 cores = 4 batches x 2 key-shards.  Core (b, h)
handles key tiles kt === h (mod 2) of batch b (2048 keys) but ALL 4096
queries, producing unnormalized partial attention sums + partial softmax
denominators; the host merges: out = (avA + avB) / (lA + lB).

v2 changes vs v1:
- All matmul operands bf16 (inputs pre-cast on host).  PSUM stays f32.
- K^T kept resident in SBUF (no DRAM round-trip; v1 re-read ~37 MB).
- No max-subtraction in softmax (|scores*scale| <~ 3), partials merge
  exactly on the host.

Per-core causal structure: for query chunk c (512 q), local key tiles
kt' = 0..2c+1 are active; the last two are diagonal and get additive
masks passed as per-core input data (mask_in[2,128,512]).
"""
import numpy as np
import ml_dtypes

import concourse.bacc as bacc
import concourse.mybir as mybir
import concourse.tile as tile
from concourse.bass_utils import run_bass_kernel_spmd

F32 = mybir.dt.float32
BF16 = mybir.dt.bfloat16
FP8 = mybir.dt.float8e4
DR = mybir.MatmulPerfMode.DoubleRow
EXP = mybir.ActivationFunctionType.Exp

B, N, E, O = 4, 4096, 1024, 1024
NE, NO = E // 128, O // 128
NK = N // 2                          # local keys per core (2048)
NKT_L = NK // 128                    # 16 local k-tiles
QC = 512
NCHUNK = N // QC                     # 8
SCALE = 1.0 / 32.0
MASKVAL = -1e9


def _emit(nc, tc, xT, xkT, WqT, WkT, WvT, mask_in, out_u, l_out):
    xT3 = xT.rearrange("(a p) n -> a p n", p=128)
    xk3 = xkT.rearrange("(a p) n -> a p n", p=128)
    WqT3 = WqT.rearrange("(a p) n -> a p n", p=128)
    WkT3 = WkT.rearrange("(a p) n -> a p n", p=128)
    WvT3 = WvT.rearrange("(a p) n -> a p n", p=128)
    out3 = out_u.rearrange("(a p) n -> a p n", p=128)

    # outer pool: resident K^T, V, Wq, masks (live across both phases)
    from contextlib import ExitStack
    _stk = ExitStack()
    outer = _stk.enter_context(tc.tile_pool(name="outer", bufs=1))
    vres = [outer.tile([128, O], BF16, tag=f"vres{k}", name=f"vres_{k}")
            for k in range(NKT_L)]
    # K^T resident as fp8 DoubleRow pairs: per (o-pair i, k-tile kt) one
    # CONTIGUOUS [128, 2, 128] stationary tile (j = o-tile 2i+j)
    kres = [[outer.tile([128, 2, 128], FP8, tag=f"kres{i}_{kt}",
                        name=f"kres_{i}_{kt}") for kt in range(NKT_L)]
            for i in range(NO // 2)]
    wq, masks = [], []
    # ---------------- phase 1: K^T (local keys) and V projections ------------
    with tc.tile_pool(name="p1", bufs=1) as sb, \
         tc.tile_pool(name="p1p", bufs=1, space="PSUM") as pp:
        # critical-path loads first: wk + x feed the first K-proj matmuls;
        # wv/wq/masks are needed later.  All 2048 local keys resident at
        # once so each wk stationary serves 4 moving blocks.
        wk, wv, xe = [], [], []
        for e in range(NE):
            wkt = sb.tile([128, O], BF16, tag=f"wk{e}", name=f"wk_{e}")
            nc.sync.dma_start(wkt, WkT3[e])
            wk.append(wkt)
            xet = sb.tile([128, NK], BF16, tag=f"xe{e}", name=f"xe_{e}")
            nc.sync.dma_start(xet, xk3[e])
            xe.append(xet)
        for e in range(NE):
            wvt = sb.tile([128, O], BF16, tag=f"wv{e}", name=f"wv_{e}")
            nc.sync.dma_start(wvt, WvT3[e])
            wv.append(wvt)
        for e in range(NE):
            wqt = outer.tile([128, O], BF16, tag=f"wq{e}", name=f"wq_{e}")
            nc.sync.dma_start(wqt, WqT3[e])
            wq.append(wqt)
        for i in range(2):
            m = outer.tile([128, QC], F32, tag=f"mask{i}", name=f"mask_{i}")
            nc.sync.dma_start(m, mask_in[i])
            masks.append(m)
        # prefetch the first chunk-pair's x during phase 1 so Q-proj can
        # start the moment the V projection ends
        xc0 = {}
        for par, c in ((0, NCHUNK - 1), (1, NCHUNK - 2)):
            xc0[par] = []
            for e in range(NE):
                t = outer.tile([128, QC], BF16, tag=f"xc0{e}_{par}",
                               name=f"xc0_{c}_{e}")
                nc.sync.dma_start(t, xT3[e][:, c * QC:(c + 1) * QC])
                xc0[par].append(t)
        for o in range(NO):
            pk = [pp.tile([128, 512], F32, tag="pp", bufs=8,
                          name=f"pk_{o}_{kc}") for kc in range(NK // 512)]
            for e in range(NE):
                for kc in range(NK // 512):
                    nc.tensor.matmul(
                        pk[kc], wk[e][:, o * 128:(o + 1) * 128],
                        xe[e][:, kc * 512:(kc + 1) * 512],
                        start=(e == 0), stop=(e == NE - 1))
            for kc in range(NK // 512):
                for t in range(4):
                    nc.scalar.copy(
                        kres[o // 2][kc * 4 + t][:, o % 2, :],
                        pk[kc][:, t * 128:(t + 1) * 128])
        for ns in range(NKT_L):
            pvs = [pp.tile([128, 512], F32, tag="pp", bufs=8,
                           name=f"pv_{ns}_{ovc}") for ovc in range(2)]
            for e in range(NE):
                for ovc in range(2):
                    nc.tensor.matmul(
                        pvs[ovc], xe[e][:, ns * 128:(ns + 1) * 128],
                        wv[e][:, ovc * 512:(ovc + 1) * 512],
                        start=(e == 0), stop=(e == NE - 1))
            for ovc in range(2):
                nc.scalar.copy(
                    vres[ns][:, ovc * 512:(ovc + 1) * 512], pvs[ovc])

    # ---------------- phase 2: attention ----------------
    with tc.tile_pool(name="p2", bufs=1) as sb, \
         tc.tile_pool(name="p2p", bufs=1, space="PSUM") as pp:
        ones = sb.tile([128, 1], F32, tag="ones", name="ones")
        nc.gpsimd.memset(ones, 1.0)

        # chunk pairs, biggest first: shared Q-proj stationaries serve
        # both chunks; big chunks' output DMA overlaps later compute
        for cp in range(NCHUNK // 2):
            cA = NCHUNK - 1 - 2 * cp
            pair = ((0, cA), (1, cA - 1))
            xc, qt = {}, {}
            for par, c in pair:
                if cp == 0:
                    xc[par] = xc0[par]      # prefetched during phase 1
                else:
                    xc[par] = []
                    for e in range(NE):
                        xct = sb.tile([128, QC], BF16, tag=f"xc{e}_{par}",
                                      bufs=1, name=f"xc_{c}_{e}")
                        nc.sync.dma_start(
                            xct, xT3[e][:, c * QC:(c + 1) * QC])
                        xc[par].append(xct)
                qt[par] = [sb.tile([128, 2, QC], FP8, tag=f"qt{i}_{par}",
                                   bufs=2, name=f"qt_{c}_{i}")
                           for i in range(NO // 2)]
            for oi in range(NO):
                qps = {par: pp.tile([128, QC], F32, tag="avqp", bufs=3,
                                    name=f"qps_{c}_{oi}")
                       for par, c in pair}
                for e in range(NE):
                    for par, c in pair:
                        nc.tensor.matmul(
                            qps[par], wq[e][:, oi * 128:(oi + 1) * 128],
                            xc[par][e],
                            start=(e == 0), stop=(e == NE - 1))
                for par, c in pair:
                    nc.scalar.copy(qt[par][oi // 2][:, oi % 2, :], qps[par])

            # scores for BOTH chunks interleaved so each kres stationary
            # serves the pair; pts kept per-parity
            pts = {}
            lacc = {}
            for par, c in pair:
                pts[par] = []
                lacc[par] = sb.tile([128, QC], F32, tag=f"lacc{par}",
                                    bufs=1, name=f"lacc_{c}")
            nktA = 2 * cA + 2
            QH = QC // 2
            for kt in range(nktA):
                # the final (di==1) diagonal k-tile is fully causal-masked
                # for queries [0, QH) at either key parity — compute scores
                # on the upper query half only
                sps, half = {}, {}
                for par, c in pair:
                    if kt >= 2 * c + 2:
                        continue
                    half[par] = kt == 2 * c + 1
                    t = pp.tile([128, QC], F32, tag="s", bufs=2,
                                name=f"sps_{c}_{kt}")
                    sps[par] = t[:, :QH] if half[par] else t
                for i in range(NO // 2):
                    for par in sps:
                        mv = (qt[par][i][:, :, QH:] if half[par]
                              else qt[par][i])
                        nc.tensor.matmul(
                            sps[par], kres[i][kt], mv,
                            start=(i == 0), stop=(i == NO // 2 - 1),
                            perf_mode=DR)
                for par, c in pair:
                    if par not in sps:
                        continue
                    nkt = 2 * c + 2
                    di = kt - (nkt - 2)
                    if half[par]:
                        nc.vector.tensor_add(sps[par], sps[par],
                                             masks[1][:, QH:])
                        pt = sb.tile([128, QH], BF16, tag=f"ptH_{par}",
                                     bufs=2, name=f"pt_{c}_{kt}")
                        nc.scalar.activation(pt, sps[par], EXP, scale=SCALE)
                        pts[par].append(pt)
                        nc.vector.tensor_add(lacc[par][:, QH:],
                                             lacc[par][:, QH:], pt)
                        continue
                    if di >= 0:
                        nc.vector.tensor_add(sps[par], sps[par], masks[di])
                    pt = sb.tile([128, QC], BF16, tag=f"pt{kt}_{par}",
                                 bufs=2, name=f"pt_{c}_{kt}")
                    nc.scalar.activation(pt, sps[par], EXP, scale=SCALE)
                    pts[par].append(pt)
                    if kt == 0:
                        nc.vector.tensor_copy(lacc[par], pt)
                    else:
                        nc.vector.tensor_add(lacc[par], lacc[par], pt)

            for par, c in pair:
                _attend(nc, sb, pp, vres, pts[par], lacc[par], ones,
                        out3, l_out, c)
    _stk.close()


def _attend(nc, sb, pp, vres, pts, lacc, ones, out3, l_out, c):
    nkt = 2 * c + 2
    if True:
        if True:
            lps = pp.tile([1, QC], F32, tag="l", bufs=1, name=f"lps_{c}")

            # AV: s-major with ovc inner so each pt stationary serves both
            # o-halves; q-subtiles s<2 of the last (diagonal) k-tile are
            # fully causal-masked for either key parity — skip exactly.
            # pts[nkt-1] is the half-width [128, QC//2] tile (queries
            # QC//2..QC), so its s-slices are offset by -2
            for s in range(4):
                last = nkt - 2 if s < 2 else nkt - 1
                av = [pp.tile([128, 512], F32, tag="av2", bufs=2,
                              name=f"av_{c}_{s}_{ovc}") for ovc in range(2)]
                for kt in range(last + 1):
                    if kt == nkt - 1:
                        stat = pts[kt][:, (s - 2) * 128:(s - 1) * 128]
                    else:
                        stat = pts[kt][:, s * 128:(s + 1) * 128]
                    for ovc in range(2):
                        nc.tensor.matmul(
                            av[ovc], stat,
                            vres[kt][:, ovc * 512:(ovc + 1) * 512],
                            start=(kt == 0), stop=(kt == last))
                for ovc in range(2):
                    ot = sb.tile([128, 512], F32, tag="ot", bufs=4,
                                 name=f"ot_{c}_{s}_{ovc}")
                    if ovc == 0:
                        nc.scalar.copy(ot, av[ovc])
                    else:
                        nc.vector.tensor_copy(ot, av[ovc])
                    nc.sync.dma_start(
                        out3[c * 4 + s][:, ovc * 512:(ovc + 1) * 512], ot)

            nc.tensor.matmul(lps, ones, lacc, start=True, stop=True)
            lt = sb.tile([1, QC], F32, tag="lt", bufs=2, name=f"lt_{c}")
            nc.scalar.copy(lt, lps)
            nc.sync.dma_start(l_out[c:c + 1, :], lt)


_NC_CACHE = None


def build_program():
    global _NC_CACHE
    if _NC_CACHE is not None:
        return _NC_CACHE
    nc = bacc.Bacc("TRN2", target_bir_lowering=False, debug=False)
    xT = nc.dram_tensor("xT", [E, N], BF16, kind="ExternalInput").ap()
    xkT = nc.dram_tensor("xkT", [E, NK], BF16, kind="ExternalInput").ap()
    WqT = nc.dram_tensor("WqT", [E, O], BF16, kind="ExternalInput").ap()
    WkT = nc.dram_tensor("WkT", [E, O], BF16, kind="ExternalInput").ap()
    WvT = nc.dram_tensor("WvT", [E, O], BF16, kind="ExternalInput").ap()
    mask_in = nc.dram_tensor("mask_in", [2, 128, QC], F32,
                             kind="ExternalInput").ap()
    out_u = nc.dram_tensor("out_u", [N, O], F32, kind="ExternalOutput").ap()
    l_out = nc.dram_tensor("l_out", [NCHUNK, QC], F32,
                           kind="ExternalOutput").ap()
    with tile.TileContext(nc) as tc:
        _emit(nc, tc, xT, xkT, WqT, WkT, WvT, mask_in, out_u, l_out)
    nc.compile()
    _NC_CACHE = nc
    return nc


def make_in_maps(x, Wq, Wk, Wv):
    bf = ml_dtypes.bfloat16
    x = np.asarray(x, np.float32)
    WqT = np.ascontiguousarray(np.asarray(Wq, np.float32).T.astype(bf))
    WkT = np.ascontiguousarray(np.asarray(Wk, np.float32).T.astype(bf))
    WvT = np.ascontiguousarray(np.asarray(Wv, np.float32).T.astype(bf))
    kk = np.arange(128)[:, None]
    qq = np.arange(QC)[None, :]
    in_maps = []
    for c in range(8):
        b, h = divmod(c, 2)
        xb = x[b]
        xk = xb.reshape(N // 128, 128, E)[h::2].reshape(NK, E)
        masks = np.stack([
            np.where(qq >= (2 * i + h) * 128 + kk, 0.0, MASKVAL)
            for i in range(2)
        ]).astype(np.float32)
        in_maps.append({
            "xT": np.ascontiguousarray(xb.T.astype(bf)),
            "xkT": np.ascontiguousarray(xk.T.astype(bf)),
            "WqT": WqT,
            "WkT": WkT,
            "WvT": WvT,
            "mask_in": masks,
        })
    return in_maps


def gather_out(results):
    out = np.empty((B, N, O), np.float32)
    for b in range(B):
        a0 = results[2 * b]["out_u"].astype(np.float64)
        a1 = results[2 * b + 1]["out_u"].astype(np.float64)
        l0 = results[2 * b]["l_out"].astype(np.float64).reshape(N, 1)
        l1 = results[2 * b + 1]["l_out"].astype(np.float64).reshape(N, 1)
        out[b] = ((a0 + a1) / (l0 + l1)).astype(np.float32)
    return out


def kernel(x, Wq, Wk, Wv, **run_kwargs):
    nc = build_program()
    in_maps = make_in_maps(x, Wq, Wk, Wv)
    res = run_bass_kernel_spmd(nc, in_maps, core_ids=list(range(8)),
                               **run_kwargs)
    out = gather_out(res.results)
    if run_kwargs:
        return out, res
    return out

